# revision 19
# baseline (speedup 1.0000x reference)
"""Trainium2 Bass kernel: ColumnParallelLinear + multi-adapter LoRA routing.

Computes out = x @ W^T + bias + B[aid[s]] @ (A[aid[s]] @ x[s]) for each token.

Distribution across 8 NeuronCores (one TRN2 chip):
  - base GEMM is tensor-parallel over d_out (sharding_hint): weight + bias
    sharded, each core emits out_base^T [512, S]
  - the LoRA delta is token-parallel: core i computes the delta for ITS
    1024-token slab across ALL d_out (A and B are tiny and replicated), so
    the rank-16 A-projection is computed once per token chip-wide instead
    of 8x replicated; no collectives needed — the host adds the two partial
    results while unsharding (out[s,o] = base[core o/512] + delta[core s/1024])
  - each core's token axis is ROTATED on the host so its own slab occupies
    the first two 512-token tiles; the xa matmuls then reuse the base
    x-strips already in SBUF (no extra x traffic, no prefetch stall), and
    the host un-rotates the base output during unsharding

Per-core kernel (all matmuls bf16, K=128 tiles, N=512 moving):
  - host pre-transposes x so the contraction dim lands on SBUF partitions
  - per-token adapter routing = precomputed {0,1} mask multiplied into the
    xa PSUM tile on the VectorE before the B_cat matmuls
  - bias is added during base PSUM->SBUF eviction (per-partition scalar add)
  - the 64 B_cat delta matmuls are drip-fed 2-per-base-m-tile so their
    PSUM-evict chain (ScalarE copy) never gates the PE
  - DMA emission is interleaved (w chunk k / x chunk k) with small leading
    chunks so the first matmul issues after ~256KB of DMA
"""

import os
import sys

import numpy as np

try:
    import ml_dtypes
except ImportError:  # pragma: no cover
    sys.path.insert(0, "/opt/trn_rl_repo")
    import ml_dtypes

_P = 128  # SBUF partitions / matmul tile edge
_NT = 512  # token tile (matmul moving free dim, one PSUM bank of fp32)
_LR = 128  # L * R = 8 * 16 adapter-rank rows
_N_CORES = 8

_NC_CACHE = {}
LAST_RESULTS = None  # BassKernelResults of the most recent run (for test.py)


def _import_concourse():
    try:
        import concourse  # noqa: F401
    except ImportError:  # pragma: no cover
        for p in ("/opt/trn_rl_repo", "/root/.axon_site/_ro/trn_rl_repo"):
            if os.path.isdir(p) and p not in sys.path:
                sys.path.insert(0, p)


def build_nc(d_in: int, d_loc: int, s_tokens: int, s_own: int, d_out: int):
    """Build + finalize the per-core Bass kernel.

    d_loc: output features of this core's base shard
    s_own: tokens in this core's LoRA-delta slab (the FIRST s_own tokens of
           the core's rotated token order)
    d_out: full output width (the delta covers all of it)
    """
    _import_concourse()
    import concourse.tile as tile
    from concourse import bacc, mybir

    P, NT, LR = _P, _NT, _LR
    n_kt = d_in // P
    n_mt = d_loc // P
    n_nt = s_tokens // NT
    n_ot = s_own // NT  # own-slab token tiles
    n_dt = d_out // P  # delta feature tiles
    assert all(v % P == 0 for v in (d_in, d_loc, d_out)) and s_tokens % NT == 0
    assert s_own % NT == 0 and n_ot <= n_nt

    nc = bacc.Bacc("TRN2", target_bir_lowering=False, debug=False)

    bf16 = mybir.dt.bfloat16
    f32 = mybir.dt.float32

    xT = nc.dram_tensor("xT", [d_in, s_tokens], bf16, kind="ExternalInput").ap()
    w_t = nc.dram_tensor("w_t", [d_in, d_loc], bf16, kind="ExternalInput").ap()
    a_t = nc.dram_tensor("a_t", [d_in, LR], bf16, kind="ExternalInput").ap()
    b_cat_t = nc.dram_tensor("b_cat_t", [LR, d_out], bf16, kind="ExternalInput").ap()
    mask_own = nc.dram_tensor("mask_own", [LR, s_own], bf16, kind="ExternalInput").ap()
    bias_pre = nc.dram_tensor("bias_pre", [P, n_mt], f32, kind="ExternalInput").ap()
    out_t = nc.dram_tensor("out_t", [d_loc, s_tokens], f32, kind="ExternalOutput").ap()
    delta_t = nc.dram_tensor("delta_t", [d_out, s_own], bf16, kind="ExternalOutput").ap()

    # [d_in, n] with d_in = kt*128 + p  ->  [p, kt, n]
    xT_v = xT.rearrange("(kt p) s -> p kt s", p=P)
    w_v = w_t.rearrange("(kt p) m -> p kt m", p=P)
    a_v = a_t.rearrange("(kt p) m -> p kt m", p=P)

    XCHUNK = 4  # k-tiles per x/w DMA chunk
    # finer chunks at the very start so the first matmul issues after ~256KB
    START_BOUNDS = [0, 1, 2, 3, 4]
    c = START_BOUNDS[-1]
    while c < n_kt:
        c = min(c + XCHUNK, n_kt)
        START_BOUNDS.append(c)
    START_BOUNDS = sorted(set(b for b in START_BOUNDS if b <= n_kt))

    with tile.TileContext(nc) as tc:
        with (
            tc.tile_pool(name="const", bufs=1) as const_pool,
            tc.tile_pool(name="xp", bufs=1) as x_pool,
            tc.tile_pool(name="outp", bufs=1) as out_pool,
            tc.tile_pool(name="psum", bufs=1, space="PSUM") as psum_pool,
        ):
            w_all = const_pool.tile([P, n_kt, d_loc], bf16)
            b_cat = const_pool.tile([P, n_dt, P], bf16)
            bias_sb = const_pool.tile([P, n_mt], f32)
            a_all = const_pool.tile([P, n_kt, LR], bf16)
            xa_sb = const_pool.tile([P, s_own], bf16)
            mask_sb = const_pool.tile([P, s_own], bf16)

            # Deferred LoRA-delta jobs, drip-fed between base m-tiles so the
            # PSUM-evict chain (ACT copy) never gates the PE.
            delta_jobs = []

            def emit_delta(k):
                for _ in range(k):
                    if not delta_jobs:
                        return
                    n, m = delta_jobs.pop(0)
                    dl_ps = psum_pool.tile(
                        [P, NT], f32, tag="dl", bufs=2, name=f"dl_ps{n}_{m}"
                    )
                    nc.tensor.matmul(
                        dl_ps[:],
                        b_cat[:, m, :],
                        xa_sb[:, n * NT : (n + 1) * NT],
                        start=True,
                        stop=True,
                    )
                    d_sb = out_pool.tile(
                        [P, NT], bf16, tag="d_sb", bufs=4, name=f"d_sb{n}_{m}"
                    )
                    nc.scalar.copy(d_sb[:], dl_ps[:])
                    nc.sync.dma_start(
                        delta_t[m * P : (m + 1) * P, n * NT : (n + 1) * NT], d_sb[:]
                    )

            def load_x_strip(j):
                x_strip = x_pool.tile(
                    [P, n_kt, NT], bf16, tag="x_strip", bufs=3, name=f"x_strip{j}"
                )
                tok = slice(j * NT, (j + 1) * NT)
                for c in range(0, n_kt, XCHUNK):
                    e = min(c + XCHUNK, n_kt)
                    nc.sync.dma_start(x_strip[:, c:e, :], xT_v[:, c:e, tok])
                return x_strip

            def evict_base(j, m, ps):
                tok0 = j * NT
                o_sb = out_pool.tile(
                    [P, NT], f32, tag="o_sb", bufs=6, name=f"o_sb{j}_{m}"
                )
                nc.vector.tensor_scalar_add(
                    out=o_sb[:], in0=ps[:], scalar1=bias_sb[:, m : m + 1]
                )
                # the very last evict+store trails the final matmul: spread it
                # over several DMA engines so the kernel tail shrinks
                pieces = {n_mt - 2: 2, n_mt - 1: 4}.get(m, 1) if j == n_nt - 1 else 1
                step = NT // pieces
                for q in range(pieces):
                    nc.sync.dma_start(
                        out_t[
                            m * P : (m + 1) * P,
                            tok0 + q * step : tok0 + (q + 1) * step,
                        ],
                        o_sb[:, q * step : (q + 1) * step],
                    )
                emit_delta(2)

            def base_ntile(j, x_strip, k_outer=False):
                if not k_outer:
                    for m in range(n_mt):
                        ps = psum_pool.tile(
                            [P, NT], f32, tag="base", bufs=4, name=f"ps{j}_{m}"
                        )
                        for kt in range(n_kt):
                            nc.tensor.matmul(
                                ps[:],
                                w_all[:, kt, m * P : (m + 1) * P],
                                x_strip[:, kt, :],
                                start=(kt == 0),
                                stop=(kt == n_kt - 1),
                            )
                        evict_base(j, m, ps)
                    return
                # k-outer: consume each k-chunk with one MM per m-tile the
                # moment it lands, so the DMA-paced prefix keeps the PE fed;
                # all n_mt PSUM banks accumulate concurrently
                pss = [
                    psum_pool.tile([P, NT], f32, tag="base", bufs=4, name=f"ps{j}_{m}")
                    for m in range(n_mt)
                ]
                for c, e in zip(START_BOUNDS, START_BOUNDS[1:]):
                    for m in range(n_mt):
                        for kt in range(c, e):
                            nc.tensor.matmul(
                                pss[m][:],
                                w_all[:, kt, m * P : (m + 1) * P],
                                x_strip[:, kt, :],
                                start=(kt == 0),
                                stop=(kt == n_kt - 1),
                            )
                for m in range(n_mt):
                    evict_base(j, m, pss[m])

            def xa_block(n, x_strip):
                # xa = A_all @ x^T for own-slab tile n, masked per-token;
                # queues that tile's 32 B_cat delta matmuls
                xa_ps = psum_pool.tile([P, NT], f32, tag="xa", bufs=2, name=f"xa_ps{n}")
                for kt in range(n_kt):
                    nc.tensor.matmul(
                        xa_ps[:],
                        a_all[:, kt, :],
                        x_strip[:, kt, :],
                        start=(kt == 0),
                        stop=(kt == n_kt - 1),
                    )
                nc.vector.tensor_mul(
                    out=xa_sb[:, n * NT : (n + 1) * NT],
                    in0=xa_ps[:],
                    in1=mask_sb[:, n * NT : (n + 1) * NT],
                )
                delta_jobs.extend((n, m) for m in range(n_dt))

            # ---- startup: interleave w chunks with x-strip j=0 chunks so the
            # first base matmuls have their operands after ~128KB of DMA;
            # the leading single-k-tile chunks are split in half across two
            # DMA engines to halve their arrival latency
            x_strip0 = x_pool.tile(
                [P, n_kt, NT], bf16, tag="x_strip", bufs=3, name="x_strip_first"
            )
            for c, e in zip(START_BOUNDS, START_BOUNDS[1:]):
                if e - c == 1:
                    q = d_loc // 4 if c == 0 else d_loc // 2
                    for h in range(0, d_loc, q):
                        nc.sync.dma_start(w_all[:, c, h : h + q], w_v[:, c, h : h + q])
                    q = NT // 4 if c == 0 else NT // 2
                    for h in range(0, NT, q):
                        nc.sync.dma_start(
                            x_strip0[:, c, h : h + q], xT_v[:, c, h : h + q]
                        )
                else:
                    nc.sync.dma_start(w_all[:, c:e, :], w_v[:, c:e, :])
                    nc.sync.dma_start(x_strip0[:, c:e, :], xT_v[:, c:e, 0:NT])
            nc.sync.dma_start(bias_sb[:], bias_pre)
            # warm the strip prefetch pipeline before any compute is emitted
            # (fresh pool slots -> these issue immediately on the Sync engine)
            strips = {0: x_strip0}
            K_OUTER = set()
            for j in (1, 2):
                if j < n_nt:
                    strips[j] = load_x_strip(j)
            # LoRA constants (a few MB; needed from ~40us in)
            for c in range(0, n_kt, XCHUNK):
                e = min(c + XCHUNK, n_kt)
                nc.sync.dma_start(a_all[:, c:e, :], a_v[:, c:e, :])
            nc.sync.dma_start(mask_sb[:], mask_own)
            for c in range(n_dt):
                nc.sync.dma_start(b_cat[:, c, :], b_cat_t[:, c * P : (c + 1) * P])

            for j in range(n_nt):
                x_strip = strips.pop(j) if j in strips else load_x_strip(j)
                base_ntile(j, x_strip, k_outer=j in K_OUTER)
                if j < n_ot:
                    xa_block(j, x_strip)
            while delta_jobs:
                emit_delta(len(delta_jobs))

    nc.finalize()
    return nc


def _get_nc(key):
    if key not in _NC_CACHE:
        _NC_CACHE[key] = build_nc(*key)
    return _NC_CACHE[key]


def make_in_maps(x, adapter_ids, weight, bias, A_buffer, B_buffer, n_cores=_N_CORES):
    """Host-side shard + layout prep. Returns (in_maps, shapes)."""
    bf16 = ml_dtypes.bfloat16
    x = np.asarray(x, dtype=np.float32)
    adapter_ids = np.asarray(adapter_ids, dtype=np.int32)
    weight = np.asarray(weight, dtype=np.float32)
    bias = np.asarray(bias, dtype=np.float32)
    A_buffer = np.asarray(A_buffer, dtype=np.float32)
    B_buffer = np.asarray(B_buffer, dtype=np.float32)

    S, D_IN = x.shape
    D_OUT = weight.shape[0]
    L, R, _ = A_buffer.shape
    d_loc = D_OUT // n_cores
    s_own = S // n_cores
    LR = L * R
    assert LR == _LR

    xT = np.ascontiguousarray(x.astype(bf16).T)  # [D_IN, S]
    a_t = np.ascontiguousarray(A_buffer.reshape(LR, D_IN).astype(bf16).T)
    b_cat_t = np.ascontiguousarray(
        B_buffer.transpose(0, 2, 1).reshape(LR, D_OUT).astype(bf16)
    )
    maskT = (np.arange(LR)[:, None] // R == adapter_ids[None, :]).astype(bf16)

    in_maps = []
    for i in range(n_cores):
        osl = slice(i * d_loc, (i + 1) * d_loc)
        w_t = np.ascontiguousarray(weight[osl].astype(bf16).T)  # [D_IN, d_loc]
        bias_pre = np.ascontiguousarray(bias[osl].reshape(d_loc // _P, _P).T)
        # rotate the token axis so core i's own slab comes first
        xT_rot = np.roll(xT, -i * s_own, axis=1) if i else xT
        in_maps.append(
            {
                "xT": np.ascontiguousarray(xT_rot),
                "w_t": w_t,
                "a_t": a_t,
                "b_cat_t": b_cat_t,
                "mask_own": np.ascontiguousarray(
                    maskT[:, i * s_own : (i + 1) * s_own]
                ),
                "bias_pre": bias_pre,
            }
        )
    return in_maps, (S, D_IN, D_OUT, d_loc, s_own)


def kernel(x, adapter_ids, weight, bias, A_buffer, B_buffer):
    global LAST_RESULTS
    _import_concourse()
    from concourse.bass_utils import run_bass_kernel_spmd

    in_maps, (S, D_IN, D_OUT, d_loc, s_own) = make_in_maps(
        x, adapter_ids, weight, bias, A_buffer, B_buffer
    )
    nc = _get_nc((D_IN, d_loc, S, s_own, D_OUT))
    LAST_RESULTS = run_bass_kernel_spmd(nc, in_maps, core_ids=list(range(_N_CORES)))
    res = LAST_RESULTS.results
    out = np.empty((S, D_OUT), dtype=np.float32)
    for i in range(_N_CORES):
        # un-rotate this core's token axis while scattering its base shard
        base = res[i]["out_t"]
        if i:
            base = np.roll(base, i * s_own, axis=1)
        out[:, i * d_loc : (i + 1) * d_loc] = base.T
    for i in range(_N_CORES):
        out[i * s_own : (i + 1) * s_own, :] += res[i]["delta_t"].T.astype(np.float32)
    return out


# revision 20
# speedup vs baseline: 1.1745x; 1.1745x over previous
"""Trainium2 Bass kernel: ColumnParallelLinear + multi-adapter LoRA routing.

Computes out = x @ W^T + bias + B[aid[s]] @ (A[aid[s]] @ x[s]) for each token.

Distribution across 8 NeuronCores (one TRN2 chip):
  - base GEMM is tensor-parallel over d_out (sharding_hint): weight + bias
    sharded, each core emits out_base^T [512, S]
  - the LoRA delta is token-parallel: core i computes the delta for ITS
    1024-token slab across ALL d_out (A and B are tiny and replicated), so
    the rank-16 A-projection is computed once per token chip-wide instead
    of 8x replicated; no collectives needed — the host adds the two partial
    results while unsharding (out[s,o] = base[core o/512] + delta[core s/1024])
  - each core's token axis is ROTATED on the host so its own slab occupies
    the first two 512-token tiles; the xa matmuls then reuse the base
    x-strips already in SBUF (no extra x traffic, no prefetch stall), and
    the host un-rotates the base output during unsharding

Per-core kernel (all matmuls bf16, K=128 tiles, N=512 moving):
  - host pre-transposes x so the contraction dim lands on SBUF partitions
  - per-token adapter routing = precomputed {0,1} mask multiplied into the
    xa PSUM tile on the VectorE before the B_cat matmuls
  - bias is added during base PSUM->SBUF eviction (per-partition scalar add)
  - the 64 B_cat delta matmuls are drip-fed 2-per-base-m-tile so their
    PSUM-evict chain (ScalarE copy) never gates the PE
  - DMA emission is interleaved (w chunk k / x chunk k) with small leading
    chunks so the first matmul issues after ~256KB of DMA
"""

import os
import sys

import numpy as np

try:
    import ml_dtypes
except ImportError:  # pragma: no cover
    sys.path.insert(0, "/opt/trn_rl_repo")
    import ml_dtypes

_P = 128  # SBUF partitions / matmul tile edge
_NT = 512  # token tile (matmul moving free dim, one PSUM bank of fp32)
_LR = 128  # L * R = 8 * 16 adapter-rank rows
_N_CORES = 8

_NC_CACHE = {}
LAST_RESULTS = None  # BassKernelResults of the most recent run (for test.py)


def _import_concourse():
    try:
        import concourse  # noqa: F401
    except ImportError:  # pragma: no cover
        for p in ("/opt/trn_rl_repo", "/root/.axon_site/_ro/trn_rl_repo"):
            if os.path.isdir(p) and p not in sys.path:
                sys.path.insert(0, p)


def build_nc(d_in: int, d_loc: int, s_tokens: int, s_own: int, d_out: int):
    """Build + finalize the per-core Bass kernel.

    d_loc: output features of this core's base shard
    s_own: tokens in this core's LoRA-delta slab (the FIRST s_own tokens of
           the core's rotated token order)
    d_out: full output width (the delta covers all of it)
    """
    _import_concourse()
    import concourse.tile as tile
    from concourse import bacc, mybir

    P, NT, LR = _P, _NT, _LR
    n_kt = d_in // P
    n_mt = d_loc // P
    n_nt = s_tokens // NT
    n_ot = s_own // NT  # own-slab token tiles
    n_dt = d_out // P  # delta feature tiles
    assert all(v % P == 0 for v in (d_in, d_loc, d_out)) and s_tokens % NT == 0
    assert s_own % NT == 0 and n_ot <= n_nt

    nc = bacc.Bacc("TRN2", target_bir_lowering=False, debug=False)

    bf16 = mybir.dt.bfloat16
    f32 = mybir.dt.float32

    xT = nc.dram_tensor("xT", [d_in, s_tokens], bf16, kind="ExternalInput").ap()
    w_t = nc.dram_tensor("w_t", [d_in, d_loc], bf16, kind="ExternalInput").ap()
    a_t = nc.dram_tensor("a_t", [d_in, LR], bf16, kind="ExternalInput").ap()
    b_cat_t = nc.dram_tensor("b_cat_t", [LR, d_out], bf16, kind="ExternalInput").ap()
    mask_own = nc.dram_tensor("mask_own", [LR, s_own], bf16, kind="ExternalInput").ap()
    bias_pre = nc.dram_tensor("bias_pre", [P, n_mt], f32, kind="ExternalInput").ap()
    out_t = nc.dram_tensor("out_t", [d_loc, s_tokens], f32, kind="ExternalOutput").ap()
    delta_t = nc.dram_tensor("delta_t", [d_out, s_own], bf16, kind="ExternalOutput").ap()

    # [d_in, n] with d_in = kt*128 + p  ->  [p, kt, n]
    xT_v = xT.rearrange("(kt p) s -> p kt s", p=P)
    w_v = w_t.rearrange("(kt p) m -> p kt m", p=P)
    a_v = a_t.rearrange("(kt p) m -> p kt m", p=P)

    XCHUNK = 4  # k-tiles per x/w DMA chunk
    # finer chunks at the very start so the first matmul issues after ~256KB
    START_BOUNDS = [0, 1, 2, 3, 4]
    c = START_BOUNDS[-1]
    while c < n_kt:
        c = min(c + XCHUNK, n_kt)
        START_BOUNDS.append(c)
    START_BOUNDS = sorted(set(b for b in START_BOUNDS if b <= n_kt))

    with tile.TileContext(nc) as tc:
        with (
            tc.tile_pool(name="const", bufs=1) as const_pool,
            tc.tile_pool(name="xp", bufs=1) as x_pool,
            tc.tile_pool(name="outp", bufs=1) as out_pool,
            tc.tile_pool(name="psum", bufs=1, space="PSUM") as psum_pool,
        ):
            w_all = const_pool.tile([P, n_kt, d_loc], bf16)
            b_cat = const_pool.tile([P, n_dt, P], bf16)
            bias_sb = const_pool.tile([P, n_mt], f32)
            a_all = const_pool.tile([P, n_kt, LR], bf16)
            xa_sb = const_pool.tile([P, s_own], bf16)
            mask_sb = const_pool.tile([P, s_own], bf16)

            # Deferred LoRA-delta jobs, drip-fed between base m-tiles so the
            # PSUM-evict chain (ACT copy) never gates the PE.
            delta_jobs = []

            def emit_delta(k):
                for _ in range(k):
                    if not delta_jobs:
                        return
                    n, m = delta_jobs.pop(0)
                    dl_ps = psum_pool.tile(
                        [P, NT], f32, tag="dl", bufs=2, name=f"dl_ps{n}_{m}"
                    )
                    nc.tensor.matmul(
                        dl_ps[:],
                        b_cat[:, m, :],
                        xa_sb[:, n * NT : (n + 1) * NT],
                        start=True,
                        stop=True,
                    )
                    d_sb = out_pool.tile(
                        [P, NT], bf16, tag="d_sb", bufs=4, name=f"d_sb{n}_{m}"
                    )
                    nc.scalar.copy(d_sb[:], dl_ps[:])
                    nc.sync.dma_start(
                        delta_t[m * P : (m + 1) * P, n * NT : (n + 1) * NT], d_sb[:]
                    )

            def load_x_strip(j):
                x_strip = x_pool.tile(
                    [P, n_kt, NT], bf16, tag="x_strip", bufs=3, name=f"x_strip{j}"
                )
                tok = slice(j * NT, (j + 1) * NT)
                for c in range(0, n_kt, XCHUNK):
                    e = min(c + XCHUNK, n_kt)
                    nc.scalar.dma_start(x_strip[:, c:e, :], xT_v[:, c:e, tok])
                return x_strip

            def evict_base(j, m, ps):
                tok0 = j * NT
                o_sb = out_pool.tile(
                    [P, NT], f32, tag="o_sb", bufs=6, name=f"o_sb{j}_{m}"
                )
                nc.vector.tensor_scalar_add(
                    out=o_sb[:], in0=ps[:], scalar1=bias_sb[:, m : m + 1]
                )
                # the very last evict+store trails the final matmul: spread it
                # over several DMA engines so the kernel tail shrinks
                pieces = {n_mt - 2: 2, n_mt - 1: 4}.get(m, 1) if j == n_nt - 1 else 1
                step = NT // pieces
                for q in range(pieces):
                    nc.sync.dma_start(
                        out_t[
                            m * P : (m + 1) * P,
                            tok0 + q * step : tok0 + (q + 1) * step,
                        ],
                        o_sb[:, q * step : (q + 1) * step],
                    )
                emit_delta(2)

            def base_ntile(j, x_strip, k_outer=False):
                if not k_outer:
                    for m in range(n_mt):
                        ps = psum_pool.tile(
                            [P, NT], f32, tag="base", bufs=4, name=f"ps{j}_{m}"
                        )
                        for kt in range(n_kt):
                            nc.tensor.matmul(
                                ps[:],
                                w_all[:, kt, m * P : (m + 1) * P],
                                x_strip[:, kt, :],
                                start=(kt == 0),
                                stop=(kt == n_kt - 1),
                            )
                        evict_base(j, m, ps)
                    return
                # k-outer: consume each k-chunk with one MM per m-tile the
                # moment it lands, so the DMA-paced prefix keeps the PE fed;
                # all n_mt PSUM banks accumulate concurrently
                pss = [
                    psum_pool.tile([P, NT], f32, tag="base", bufs=4, name=f"ps{j}_{m}")
                    for m in range(n_mt)
                ]
                for c, e in zip(START_BOUNDS, START_BOUNDS[1:]):
                    for m in range(n_mt):
                        for kt in range(c, e):
                            nc.tensor.matmul(
                                pss[m][:],
                                w_all[:, kt, m * P : (m + 1) * P],
                                x_strip[:, kt, :],
                                start=(kt == 0),
                                stop=(kt == n_kt - 1),
                            )
                for m in range(n_mt):
                    evict_base(j, m, pss[m])

            def xa_block(n, x_strip):
                # xa = A_all @ x^T for own-slab tile n, masked per-token;
                # queues that tile's 32 B_cat delta matmuls
                xa_ps = psum_pool.tile([P, NT], f32, tag="xa", bufs=2, name=f"xa_ps{n}")
                for kt in range(n_kt):
                    nc.tensor.matmul(
                        xa_ps[:],
                        a_all[:, kt, :],
                        x_strip[:, kt, :],
                        start=(kt == 0),
                        stop=(kt == n_kt - 1),
                    )
                nc.vector.tensor_mul(
                    out=xa_sb[:, n * NT : (n + 1) * NT],
                    in0=xa_ps[:],
                    in1=mask_sb[:, n * NT : (n + 1) * NT],
                )
                delta_jobs.extend((n, m) for m in range(n_dt))

            # ---- startup: interleave w chunks with x-strip j=0 chunks so the
            # first base matmuls have their operands after ~128KB of DMA;
            # the leading single-k-tile chunks are split in half across two
            # DMA engines to halve their arrival latency
            x_strip0 = x_pool.tile(
                [P, n_kt, NT], bf16, tag="x_strip", bufs=3, name="x_strip_first"
            )
            for c, e in zip(START_BOUNDS, START_BOUNDS[1:]):
                if e - c == 1:
                    q = d_loc // 4 if c == 0 else d_loc // 2
                    for h in range(0, d_loc, q):
                        nc.sync.dma_start(w_all[:, c, h : h + q], w_v[:, c, h : h + q])
                    q = NT // 4 if c == 0 else NT // 2
                    for h in range(0, NT, q):
                        nc.scalar.dma_start(
                            x_strip0[:, c, h : h + q], xT_v[:, c, h : h + q]
                        )
                else:
                    nc.sync.dma_start(w_all[:, c:e, :], w_v[:, c:e, :])
                    nc.scalar.dma_start(x_strip0[:, c:e, :], xT_v[:, c:e, 0:NT])
            nc.sync.dma_start(bias_sb[:], bias_pre)
            # warm the strip prefetch pipeline before any compute is emitted
            # (fresh pool slots -> these issue immediately on the Sync engine)
            strips = {0: x_strip0}
            K_OUTER = set()
            for j in (1, 2):
                if j < n_nt:
                    strips[j] = load_x_strip(j)
            # LoRA constants (a few MB; needed from ~40us in)
            for c in range(0, n_kt, XCHUNK):
                e = min(c + XCHUNK, n_kt)
                nc.sync.dma_start(a_all[:, c:e, :], a_v[:, c:e, :])
            nc.sync.dma_start(mask_sb[:], mask_own)
            for c in range(n_dt):
                nc.sync.dma_start(b_cat[:, c, :], b_cat_t[:, c * P : (c + 1) * P])

            for j in range(n_nt):
                x_strip = strips.pop(j) if j in strips else load_x_strip(j)
                base_ntile(j, x_strip, k_outer=j in K_OUTER)
                if j < n_ot:
                    xa_block(j, x_strip)
            while delta_jobs:
                emit_delta(len(delta_jobs))

    nc.finalize()
    return nc


def _get_nc(key):
    if key not in _NC_CACHE:
        _NC_CACHE[key] = build_nc(*key)
    return _NC_CACHE[key]


def make_in_maps(x, adapter_ids, weight, bias, A_buffer, B_buffer, n_cores=_N_CORES):
    """Host-side shard + layout prep. Returns (in_maps, shapes)."""
    bf16 = ml_dtypes.bfloat16
    x = np.asarray(x, dtype=np.float32)
    adapter_ids = np.asarray(adapter_ids, dtype=np.int32)
    weight = np.asarray(weight, dtype=np.float32)
    bias = np.asarray(bias, dtype=np.float32)
    A_buffer = np.asarray(A_buffer, dtype=np.float32)
    B_buffer = np.asarray(B_buffer, dtype=np.float32)

    S, D_IN = x.shape
    D_OUT = weight.shape[0]
    L, R, _ = A_buffer.shape
    d_loc = D_OUT // n_cores
    s_own = S // n_cores
    LR = L * R
    assert LR == _LR

    xT = np.ascontiguousarray(x.astype(bf16).T)  # [D_IN, S]
    a_t = np.ascontiguousarray(A_buffer.reshape(LR, D_IN).astype(bf16).T)
    b_cat_t = np.ascontiguousarray(
        B_buffer.transpose(0, 2, 1).reshape(LR, D_OUT).astype(bf16)
    )
    maskT = (np.arange(LR)[:, None] // R == adapter_ids[None, :]).astype(bf16)

    in_maps = []
    for i in range(n_cores):
        osl = slice(i * d_loc, (i + 1) * d_loc)
        w_t = np.ascontiguousarray(weight[osl].astype(bf16).T)  # [D_IN, d_loc]
        bias_pre = np.ascontiguousarray(bias[osl].reshape(d_loc // _P, _P).T)
        # rotate the token axis so core i's own slab comes first
        xT_rot = np.roll(xT, -i * s_own, axis=1) if i else xT
        in_maps.append(
            {
                "xT": np.ascontiguousarray(xT_rot),
                "w_t": w_t,
                "a_t": a_t,
                "b_cat_t": b_cat_t,
                "mask_own": np.ascontiguousarray(
                    maskT[:, i * s_own : (i + 1) * s_own]
                ),
                "bias_pre": bias_pre,
            }
        )
    return in_maps, (S, D_IN, D_OUT, d_loc, s_own)


def kernel(x, adapter_ids, weight, bias, A_buffer, B_buffer):
    global LAST_RESULTS
    _import_concourse()
    from concourse.bass_utils import run_bass_kernel_spmd

    in_maps, (S, D_IN, D_OUT, d_loc, s_own) = make_in_maps(
        x, adapter_ids, weight, bias, A_buffer, B_buffer
    )
    nc = _get_nc((D_IN, d_loc, S, s_own, D_OUT))
    LAST_RESULTS = run_bass_kernel_spmd(nc, in_maps, core_ids=list(range(_N_CORES)))
    res = LAST_RESULTS.results
    out = np.empty((S, D_OUT), dtype=np.float32)
    for i in range(_N_CORES):
        # un-rotate this core's token axis while scattering its base shard
        base = res[i]["out_t"]
        if i:
            base = np.roll(base, i * s_own, axis=1)
        out[:, i * d_loc : (i + 1) * d_loc] = base.T
    for i in range(_N_CORES):
        out[i * s_own : (i + 1) * s_own, :] += res[i]["delta_t"].T.astype(np.float32)
    return out


# revision 21
# speedup vs baseline: 1.1971x; 1.0192x over previous
"""Trainium2 Bass kernel: ColumnParallelLinear + multi-adapter LoRA routing.

Computes out = x @ W^T + bias + B[aid[s]] @ (A[aid[s]] @ x[s]) for each token.

Distribution across 8 NeuronCores (one TRN2 chip):
  - base GEMM is tensor-parallel over d_out (sharding_hint): weight + bias
    sharded, each core emits out_base^T [512, S]
  - the LoRA delta is token-parallel: core i computes the delta for ITS
    1024-token slab across ALL d_out (A and B are tiny and replicated), so
    the rank-16 A-projection is computed once per token chip-wide instead
    of 8x replicated; no collectives needed — the host adds the two partial
    results while unsharding (out[s,o] = base[core o/512] + delta[core s/1024])
  - each core's token axis is ROTATED on the host so its own slab occupies
    the first two 512-token tiles; the xa matmuls then reuse the base
    x-strips already in SBUF (no extra x traffic, no prefetch stall), and
    the host un-rotates the base output during unsharding

Per-core kernel (all matmuls bf16, K=128 tiles, N=512 moving):
  - host pre-transposes x so the contraction dim lands on SBUF partitions
  - per-token adapter routing = precomputed {0,1} mask multiplied into the
    xa PSUM tile on the VectorE before the B_cat matmuls
  - bias is added during base PSUM->SBUF eviction (per-partition scalar add)
  - the 64 B_cat delta matmuls are drip-fed 2-per-base-m-tile so their
    PSUM-evict chain (ScalarE copy) never gates the PE
  - DMA emission is interleaved (w chunk k / x chunk k) with small leading
    chunks so the first matmul issues after ~256KB of DMA
"""

import os
import sys

import numpy as np

try:
    import ml_dtypes
except ImportError:  # pragma: no cover
    sys.path.insert(0, "/opt/trn_rl_repo")
    import ml_dtypes

_P = 128  # SBUF partitions / matmul tile edge
_NT = 512  # token tile (matmul moving free dim, one PSUM bank of fp32)
_LR = 128  # L * R = 8 * 16 adapter-rank rows
_N_CORES = 8

_NC_CACHE = {}
LAST_RESULTS = None  # BassKernelResults of the most recent run (for test.py)


def _import_concourse():
    try:
        import concourse  # noqa: F401
    except ImportError:  # pragma: no cover
        for p in ("/opt/trn_rl_repo", "/root/.axon_site/_ro/trn_rl_repo"):
            if os.path.isdir(p) and p not in sys.path:
                sys.path.insert(0, p)


def build_nc(d_in: int, d_loc: int, s_tokens: int, s_own: int, d_out: int):
    """Build + finalize the per-core Bass kernel.

    d_loc: output features of this core's base shard
    s_own: tokens in this core's LoRA-delta slab (the FIRST s_own tokens of
           the core's rotated token order)
    d_out: full output width (the delta covers all of it)
    """
    _import_concourse()
    import concourse.tile as tile
    from concourse import bacc, mybir

    P, NT, LR = _P, _NT, _LR
    n_kt = d_in // P
    n_mt = d_loc // P
    n_nt = s_tokens // NT
    n_ot = s_own // NT  # own-slab token tiles
    n_dt = d_out // P  # delta feature tiles
    assert all(v % P == 0 for v in (d_in, d_loc, d_out)) and s_tokens % NT == 0
    assert s_own % NT == 0 and n_ot <= n_nt

    nc = bacc.Bacc("TRN2", target_bir_lowering=False, debug=False)

    bf16 = mybir.dt.bfloat16
    f32 = mybir.dt.float32

    xT = nc.dram_tensor("xT", [d_in, s_tokens], bf16, kind="ExternalInput").ap()
    w_t = nc.dram_tensor("w_t", [d_in, d_loc], bf16, kind="ExternalInput").ap()
    a_t = nc.dram_tensor("a_t", [d_in, LR], bf16, kind="ExternalInput").ap()
    b_cat_t = nc.dram_tensor("b_cat_t", [LR, d_out], bf16, kind="ExternalInput").ap()
    mask_own = nc.dram_tensor("mask_own", [LR, s_own], bf16, kind="ExternalInput").ap()
    bias_pre = nc.dram_tensor("bias_pre", [P, n_mt], f32, kind="ExternalInput").ap()
    out_t = nc.dram_tensor("out_t", [d_loc, s_tokens], f32, kind="ExternalOutput").ap()
    delta_t = nc.dram_tensor("delta_t", [d_out, s_own], bf16, kind="ExternalOutput").ap()

    # [d_in, n] with d_in = kt*128 + p  ->  [p, kt, n]
    xT_v = xT.rearrange("(kt p) s -> p kt s", p=P)
    w_v = w_t.rearrange("(kt p) m -> p kt m", p=P)
    a_v = a_t.rearrange("(kt p) m -> p kt m", p=P)

    XCHUNK = 4  # k-tiles per x/w DMA chunk
    # finer chunks at the very start so the first matmul issues after ~256KB
    START_BOUNDS = [0, 1, 2, 3, 4]
    c = START_BOUNDS[-1]
    while c < n_kt:
        c = min(c + XCHUNK, n_kt)
        START_BOUNDS.append(c)
    START_BOUNDS = sorted(set(b for b in START_BOUNDS if b <= n_kt))

    with tile.TileContext(nc) as tc:
        with (
            tc.tile_pool(name="const", bufs=1) as const_pool,
            tc.tile_pool(name="xp", bufs=1) as x_pool,
            tc.tile_pool(name="outp", bufs=1) as out_pool,
            tc.tile_pool(name="psum", bufs=1, space="PSUM") as psum_pool,
        ):
            w_all = const_pool.tile([P, n_kt, d_loc], bf16)
            b_cat = const_pool.tile([P, n_dt, P], bf16)
            bias_sb = const_pool.tile([P, n_mt], f32)
            a_all = const_pool.tile([P, n_kt, LR], bf16)
            xa_sb = const_pool.tile([P, s_own], bf16)
            mask_sb = const_pool.tile([P, s_own], bf16)

            # Deferred LoRA-delta jobs, drip-fed between base m-tiles so the
            # PSUM-evict chain (ACT copy) never gates the PE.
            delta_jobs = []

            def emit_delta(k):
                for _ in range(k):
                    if not delta_jobs:
                        return
                    n, m = delta_jobs.pop(0)
                    dl_ps = psum_pool.tile(
                        [P, NT], f32, tag="dl", bufs=2, name=f"dl_ps{n}_{m}"
                    )
                    nc.tensor.matmul(
                        dl_ps[:],
                        b_cat[:, m, :],
                        xa_sb[:, n * NT : (n + 1) * NT],
                        start=True,
                        stop=True,
                    )
                    d_sb = out_pool.tile(
                        [P, NT], bf16, tag="d_sb", bufs=4, name=f"d_sb{n}_{m}"
                    )
                    nc.scalar.copy(d_sb[:], dl_ps[:])
                    nc.sync.dma_start(
                        delta_t[m * P : (m + 1) * P, n * NT : (n + 1) * NT], d_sb[:]
                    )

            def load_x_strip(j):
                x_strip = x_pool.tile(
                    [P, n_kt, NT], bf16, tag="x_strip", bufs=3, name=f"x_strip{j}"
                )
                tok = slice(j * NT, (j + 1) * NT)
                for c in range(0, n_kt, XCHUNK):
                    e = min(c + XCHUNK, n_kt)
                    nc.sync.dma_start(x_strip[:, c:e, :], xT_v[:, c:e, tok])
                return x_strip

            def evict_base(j, m, ps):
                tok0 = j * NT
                o_sb = out_pool.tile(
                    [P, NT], f32, tag="o_sb", bufs=6, name=f"o_sb{j}_{m}"
                )
                nc.vector.tensor_scalar_add(
                    out=o_sb[:], in0=ps[:], scalar1=bias_sb[:, m : m + 1]
                )
                # the very last evict+store trails the final matmul: spread it
                # over several DMA engines so the kernel tail shrinks
                pieces = {n_mt - 2: 2, n_mt - 1: 4}.get(m, 1) if j == n_nt - 1 else 1
                step = NT // pieces
                for q in range(pieces):
                    nc.sync.dma_start(
                        out_t[
                            m * P : (m + 1) * P,
                            tok0 + q * step : tok0 + (q + 1) * step,
                        ],
                        o_sb[:, q * step : (q + 1) * step],
                    )
                emit_delta(2)

            def base_ntile(j, x_strip, k_outer=False):
                if not k_outer:
                    for m in range(n_mt):
                        ps = psum_pool.tile(
                            [P, NT], f32, tag="base", bufs=4, name=f"ps{j}_{m}"
                        )
                        for kt in range(n_kt):
                            nc.tensor.matmul(
                                ps[:],
                                w_all[:, kt, m * P : (m + 1) * P],
                                x_strip[:, kt, :],
                                start=(kt == 0),
                                stop=(kt == n_kt - 1),
                            )
                        evict_base(j, m, ps)
                    return
                # k-outer: consume each k-chunk with one MM per m-tile the
                # moment it lands, so the DMA-paced prefix keeps the PE fed;
                # all n_mt PSUM banks accumulate concurrently
                pss = [
                    psum_pool.tile([P, NT], f32, tag="base", bufs=4, name=f"ps{j}_{m}")
                    for m in range(n_mt)
                ]
                for c, e in zip(START_BOUNDS, START_BOUNDS[1:]):
                    for m in range(n_mt):
                        for kt in range(c, e):
                            nc.tensor.matmul(
                                pss[m][:],
                                w_all[:, kt, m * P : (m + 1) * P],
                                x_strip[:, kt, :],
                                start=(kt == 0),
                                stop=(kt == n_kt - 1),
                            )
                for m in range(n_mt):
                    evict_base(j, m, pss[m])

            def xa_block(n, x_strip):
                # xa = A_all @ x^T for own-slab tile n, masked per-token;
                # queues that tile's 32 B_cat delta matmuls
                xa_ps = psum_pool.tile([P, NT], f32, tag="xa", bufs=2, name=f"xa_ps{n}")
                for kt in range(n_kt):
                    nc.tensor.matmul(
                        xa_ps[:],
                        a_all[:, kt, :],
                        x_strip[:, kt, :],
                        start=(kt == 0),
                        stop=(kt == n_kt - 1),
                    )
                nc.vector.tensor_mul(
                    out=xa_sb[:, n * NT : (n + 1) * NT],
                    in0=xa_ps[:],
                    in1=mask_sb[:, n * NT : (n + 1) * NT],
                )
                delta_jobs.extend((n, m) for m in range(n_dt))

            # ---- startup: interleave w chunks with x-strip j=0 chunks so the
            # first base matmuls have their operands after ~128KB of DMA;
            # the leading single-k-tile chunks are split in half across two
            # DMA engines to halve their arrival latency
            x_strip0 = x_pool.tile(
                [P, n_kt, NT], bf16, tag="x_strip", bufs=3, name="x_strip_first"
            )
            for c, e in zip(START_BOUNDS, START_BOUNDS[1:]):
                if e - c == 1:
                    q = d_loc // 4 if c == 0 else d_loc // 2
                    for h in range(0, d_loc, q):
                        nc.sync.dma_start(w_all[:, c, h : h + q], w_v[:, c, h : h + q])
                    q = NT // 4 if c == 0 else NT // 2
                    for h in range(0, NT, q):
                        nc.sync.dma_start(
                            x_strip0[:, c, h : h + q], xT_v[:, c, h : h + q]
                        )
                else:
                    nc.sync.dma_start(w_all[:, c:e, :], w_v[:, c:e, :])
                    nc.sync.dma_start(x_strip0[:, c:e, :], xT_v[:, c:e, 0:NT])
            nc.sync.dma_start(bias_sb[:], bias_pre)
            # warm the strip prefetch pipeline before any compute is emitted
            # (fresh pool slots -> these issue immediately on the Sync engine)
            strips = {0: x_strip0}
            K_OUTER = set()
            for j in (1, 2):
                if j < n_nt:
                    strips[j] = load_x_strip(j)
            # LoRA constants (a few MB; needed from ~40us in)
            for c in range(0, n_kt, XCHUNK):
                e = min(c + XCHUNK, n_kt)
                nc.sync.dma_start(a_all[:, c:e, :], a_v[:, c:e, :])
            nc.sync.dma_start(mask_sb[:], mask_own)
            for c in range(n_dt):
                nc.sync.dma_start(b_cat[:, c, :], b_cat_t[:, c * P : (c + 1) * P])

            for j in range(n_nt):
                x_strip = strips.pop(j) if j in strips else load_x_strip(j)
                base_ntile(j, x_strip, k_outer=j in K_OUTER)
                if j < n_ot:
                    xa_block(j, x_strip)
            while delta_jobs:
                emit_delta(len(delta_jobs))

    nc.finalize()
    return nc


def _get_nc(key):
    if key not in _NC_CACHE:
        _NC_CACHE[key] = build_nc(*key)
    return _NC_CACHE[key]


def make_in_maps(x, adapter_ids, weight, bias, A_buffer, B_buffer, n_cores=_N_CORES):
    """Host-side shard + layout prep. Returns (in_maps, shapes)."""
    bf16 = ml_dtypes.bfloat16
    x = np.asarray(x, dtype=np.float32)
    adapter_ids = np.asarray(adapter_ids, dtype=np.int32)
    weight = np.asarray(weight, dtype=np.float32)
    bias = np.asarray(bias, dtype=np.float32)
    A_buffer = np.asarray(A_buffer, dtype=np.float32)
    B_buffer = np.asarray(B_buffer, dtype=np.float32)

    S, D_IN = x.shape
    D_OUT = weight.shape[0]
    L, R, _ = A_buffer.shape
    d_loc = D_OUT // n_cores
    s_own = S // n_cores
    LR = L * R
    assert LR == _LR

    xT = np.ascontiguousarray(x.astype(bf16).T)  # [D_IN, S]
    a_t = np.ascontiguousarray(A_buffer.reshape(LR, D_IN).astype(bf16).T)
    b_cat_t = np.ascontiguousarray(
        B_buffer.transpose(0, 2, 1).reshape(LR, D_OUT).astype(bf16)
    )
    maskT = (np.arange(LR)[:, None] // R == adapter_ids[None, :]).astype(bf16)

    in_maps = []
    for i in range(n_cores):
        osl = slice(i * d_loc, (i + 1) * d_loc)
        w_t = np.ascontiguousarray(weight[osl].astype(bf16).T)  # [D_IN, d_loc]
        bias_pre = np.ascontiguousarray(bias[osl].reshape(d_loc // _P, _P).T)
        # rotate the token axis so core i's own slab comes first
        xT_rot = np.roll(xT, -i * s_own, axis=1) if i else xT
        in_maps.append(
            {
                "xT": np.ascontiguousarray(xT_rot),
                "w_t": w_t,
                "a_t": a_t,
                "b_cat_t": b_cat_t,
                "mask_own": np.ascontiguousarray(
                    maskT[:, i * s_own : (i + 1) * s_own]
                ),
                "bias_pre": bias_pre,
            }
        )
    return in_maps, (S, D_IN, D_OUT, d_loc, s_own)


def kernel(x, adapter_ids, weight, bias, A_buffer, B_buffer):
    global LAST_RESULTS
    _import_concourse()
    from concourse.bass_utils import run_bass_kernel_spmd

    in_maps, (S, D_IN, D_OUT, d_loc, s_own) = make_in_maps(
        x, adapter_ids, weight, bias, A_buffer, B_buffer
    )
    nc = _get_nc((D_IN, d_loc, S, s_own, D_OUT))
    LAST_RESULTS = run_bass_kernel_spmd(nc, in_maps, core_ids=list(range(_N_CORES)))
    res = LAST_RESULTS.results
    out = np.empty((S, D_OUT), dtype=np.float32)
    for i in range(_N_CORES):
        # un-rotate this core's token axis while scattering its base shard
        base = res[i]["out_t"]
        if i:
            base = np.roll(base, i * s_own, axis=1)
        out[:, i * d_loc : (i + 1) * d_loc] = base.T
    for i in range(_N_CORES):
        out[i * s_own : (i + 1) * s_own, :] += res[i]["delta_t"].T.astype(np.float32)
    return out
